# revision 2
# baseline (speedup 1.0000x reference)
"""Trainium2 Bass kernel for nn_BigFanoutModel (100 tiny fanout matmuls + sum).

Math: out[k] = sum_{n,d} x[0,d] * matrices[n,d,k] == x @ (sum_n matrices[n]).
Shapes: x (1,4) f32, matrices (100,4,4) f32 -> out (4,) f32.

Total input is 6.4KB, so the problem is pure latency. Per the sharding hint
("too small to shard meaningfully"), the full inputs are replicated on all 8
cores; every core computes the full output with a minimal instruction chain
and core 0's result is returned. No collectives.

Per-core dataflow (engines: SP=sync DMA, DVE=vector, PE=tensor):
  SP   A_sb[25,64]  <- matrices, contiguous (25 rows x 256B descriptors)
  SP   x_sb[1,4]    <- x
  DVE  ones[25,1]   <- memset 1.0
  DVE  T[25,16]     <- sum over the 4 matrices within each row-group
                       (A viewed per-partition as [dk=16, n''=4], reduce X)
  PE   U[1,16]      <- ones.T @ T      (contracts the 25 row-groups)
  DVE  W[1,16]      <- U * x  (x broadcast along k via stride-0 AP)
  DVE  res[1,4]     <- sum over d of W (strided view, reduce X)
  SP   out[4]       <- res   (completion covered by the NEFF-end SP drain;
                              no engine stalls on the ~1us HBM write receipt)

Implementation notes:
- Raw Bass (no Tile): the whole kernel is 9 instructions; Tile's scheduler
  and its kernel-tail barrier only add overhead at this size.
- "Lean" Bass construction: the const-AP memsets and the init-time
  all-engine barrier emitted by Bass.__init__ are suppressed (nothing here
  uses the const pool, and the NEFF's own prologue already synchronizes the
  engines). No Block() wrapper -> no exit barrier.
- The DVE mul->reduce pair carries an explicit same-engine semaphore wait:
  DVE pipelines back-to-back instructions, so the reduce would otherwise
  read w_sb before the multiply's writes land (confirmed by the race
  detector in CoreSim and by a wrong result on hardware).
- fp32 matmul runs as a LOW/HIGH dual pass on the PE; keeping N=16 makes
  that pass pair ~180ns each instead of ~850ns at N=400.
"""

import numpy as np

import concourse.bass as bass
import concourse.mybir as mybir
from concourse.bass_utils import run_bass_kernel_spmd

N_CORES = 8

_NC_CACHE = None


def _make_bass_lean():
    """Bass() without the const-AP memsets and init all-engine barrier."""
    orig_barrier = bass.Bass.all_engine_barrier
    orig_memset = bass.BassGpSimd.memset
    bass.Bass.all_engine_barrier = lambda self, **k: None
    bass.BassGpSimd.memset = lambda self, ap, c: None
    try:
        nc = bass.Bass(monotonic_sem_count=0)
    finally:
        bass.Bass.all_engine_barrier = orig_barrier
        bass.BassGpSimd.memset = orig_memset
    return nc


def _build_nc():
    nc = _make_bass_lean()
    x = nc.dram_tensor("x", [1, 4], mybir.dt.float32, kind="ExternalInput")
    m = nc.dram_tensor("matrices", [100, 4, 4], mybir.dt.float32, kind="ExternalInput")
    o = nc.dram_tensor("out", [4], mybir.dt.float32, kind="ExternalOutput")
    with (
        nc.semaphore("semA") as semA,
        nc.semaphore("semX") as semX,
        nc.semaphore("semO") as semO,
        nc.semaphore("c") as c,
        nc.sbuf_tensor("A_sb", [25, 64], mybir.dt.float32) as A_sb,
        nc.sbuf_tensor("T_sb", [25, 16], mybir.dt.float32) as T_sb,
        nc.sbuf_tensor("ones_sb", [25, 1], mybir.dt.float32) as ones_sb,
        nc.sbuf_tensor("x_sb", [1, 4], mybir.dt.float32) as x_sb,
        nc.sbuf_tensor("w_sb", [1, 16], mybir.dt.float32) as w_sb,
        nc.sbuf_tensor("res_sb", [1, 4], mybir.dt.float32) as res_sb,
        nc.psum_tensor("u_ps", [1, 16], mybir.dt.float32) as u_ps,
    ):
        # SP: A first (its receipt is the long pole), x second.
        nc.sync.dma_start(
            bass.AP(A_sb, 0, [[64, 25], [1, 64]]),
            bass.AP(m, 0, [[64, 25], [1, 64]]),
        ).then_inc(semA, 16)
        nc.sync.dma_start(
            bass.AP(x_sb, 0, [[4, 1], [1, 4]]),
            bass.AP(x, 0, [[4, 1], [1, 4]]),
        ).then_inc(semX, 16)

        # DVE: ones; then T[p, (d,k)] = sum_{n''} A[p, n''*16 + (d,k)]
        nc.vector.memset(bass.AP(ones_sb, 0, [[1, 25], [1, 1]]), 1.0).then_inc(c, 1)
        nc.vector.wait_ge(semA, 16)
        nc.vector.reduce_sum(
            out=bass.AP(T_sb, 0, [[16, 25], [1, 16]]),
            in_=bass.AP(A_sb, 0, [[64, 25], [1, 16], [16, 4]]),
            axis=mybir.AxisListType.X,
        ).then_inc(c, 1)

        # PE: U[1,16] = ones.T @ T
        nc.tensor.wait_ge(c, 2)
        nc.tensor.matmul(
            bass.AP(u_ps, 0, [[16, 1], [1, 16]]),
            bass.AP(ones_sb, 0, [[1, 25], [1, 1]]),
            bass.AP(T_sb, 0, [[16, 25], [1, 16]]),
        ).then_inc(c, 1)

        # DVE: W[d,k] = U[d,k] * x[d]; then res[k] = sum_d W[d,k]
        nc.vector.wait_ge(c, 3)
        nc.vector.wait_ge(semX, 16)
        nc.vector.tensor_mul(
            bass.AP(w_sb, 0, [[16, 1], [4, 4], [1, 4]]),
            bass.AP(u_ps, 0, [[16, 1], [4, 4], [1, 4]]),
            bass.AP(x_sb, 0, [[4, 1], [1, 4], [0, 4]]),
        ).then_inc(c, 1)
        nc.vector.wait_ge(c, 4)  # same-engine pipeline hazard on w_sb
        nc.vector.reduce_sum(
            out=bass.AP(res_sb, 0, [[4, 1], [1, 4]]),
            in_=bass.AP(w_sb, 0, [[16, 1], [1, 4], [4, 4]]),
            axis=mybir.AxisListType.X,
        ).then_inc(c, 1)

        # SP: out. semO is incremented on completion but never waited on:
        # the NEFF-end SP drain retires the DMA before the NEFF completes,
        # so no engine stalls on the HBM write receipt.
        nc.sync.wait_ge(c, 5)
        nc.sync.dma_start(
            bass.AP(o, 0, [[1, 4]]),
            bass.AP(res_sb, 0, [[4, 1], [1, 4]]),
        ).then_inc(semO, 16)
    return nc


def _get_nc():
    global _NC_CACHE
    if _NC_CACHE is None:
        _NC_CACHE = _build_nc()
    return _NC_CACHE


def _run(x, matrices, **kwargs):
    nc = _get_nc()
    in_map = {
        "x": np.ascontiguousarray(x, dtype=np.float32),
        "matrices": np.ascontiguousarray(matrices, dtype=np.float32),
    }
    in_maps = [in_map for _ in range(N_CORES)]
    return run_bass_kernel_spmd(nc, in_maps, list(range(N_CORES)), **kwargs)


def kernel(x, matrices):
    res = _run(x, matrices)
    return np.asarray(res.results[0]["out"], dtype=np.float32).reshape(4)
